# revision 11
# baseline (speedup 1.0000x reference)
import sys
import json
import math

sys.path.insert(0, "/opt/trn_rl_repo")

import numpy as np
from operator import add as _op_add
from contextlib import ExitStack

import concourse.bass as bass
import concourse.tile as tile
from concourse import mybir
from concourse import dve_ops
from concourse.dve_spec import (
    Spec, Bin, AluOp, Src0, Src1, C0, C1, Zero, One, maxx, lower,
)
from concourse.dve_uop import DveOpSpec
from concourse.bass_utils import run_bass_kernel_spmd

F = np.float32
PI = math.pi
dt = mybir.dt
Alu = mybir.AluOpType
Act = mybir.ActivationFunctionType

N_CORES = 8
B_TOTAL = 32768
B_CORE = B_TOTAL // N_CORES  # 4096
P = 128
N_TILES = B_CORE // P  # 32
NF = 1086  # flat width of gradient slabs ([32,34] layout, cols 32/33 junk)

EPS8 = float(F(8.0) * F(1e-8))
EPS64 = float(F(64.0) * F(1e-8))
C18PI = float(F(18.0 / PI))
CNPI18 = float(F(-PI / 18.0))
CPI = float(F(PI))


# ---------- walrus 1-wait-per-inst workaround ----------
def _split_multiwait(data):
    n = 0
    for f in data.get("functions", []):
        for blk in f.get("blocks", []):
            insts = blk.get("instructions")
            if not insts:
                continue
            out = []
            for inst in insts:
                si = inst.get("sync_info")
                waits = (si or {}).get("on_wait") or []
                if len(waits) > 1:
                    for w in waits[:-1]:
                        n += 1
                        out.append({
                            "debug": inst.get("debug", 0),
                            "engine": inst["engine"],
                            "ins": [],
                            "name": f"{inst['name']}-sw{n}",
                            "opcode": "NoOp",
                            "outs": [],
                            "sync_info": {"on_update": [], "on_wait": [w]},
                        })
                    si["on_wait"] = [waits[-1]]
                out.append(inst)
            blk["instructions"] = out
    return n


def _install_birpatch():
    if getattr(bass.Bass.to_json_bytes, "_multiwait_patched", False):
        return
    orig = bass.Bass.to_json_bytes

    def patched(self):
        data = json.loads(orig(self))
        _split_multiwait(data)
        return json.dumps(data).encode()

    patched._multiwait_patched = True
    bass.Bass.to_json_bytes = patched


# ---------- custom DVE ops ----------
def _register(name, body, accum=None, accum_init=None, ref=None):
    if name in dve_ops._SUB_OPCODE_FOR_NAME:
        return next(o for o in dve_ops.OPS if o.name == name)
    spec = Spec(body=body, accum=accum, accum_init=accum_init, reference=ref)
    row = dve_ops._CUSTOM_DVE_ROW_BASE + len(dve_ops.OPS)
    dve_ops._SUB_OPCODE_FOR_NAME[name] = row
    sha = DveOpSpec(name=name, opcode=row,
                    uops=lower(spec, ver="v3"), rd1_en=True).sha("v3")
    op = dve_ops.DveOp(name, spec, subdim=False, uops_sha={"v3": sha})
    dve_ops.OPS.append(op)
    dve_ops.CUSTOM_DVE_SPECS[name] = spec
    return op


def _hat_ref(in0, in1, s0, s1, imm2):
    b = (np.maximum(1.0 - np.abs(in0.astype(np.float32) - s0), 0.0) * in1)
    b = b.astype(np.float32)
    return b, s1 + b.reshape(b.shape[0], -1).sum(axis=-1, keepdims=True)


def _sqad_ref(in0, in1, s0, s1, imm2):
    a = in0.astype(np.float32)
    b = in1.astype(np.float32)
    return (a * a + b * b).astype(np.float32), None


def _hat2_ref(in0, in1, s0, s1, imm2):
    o = in0.astype(np.float32)
    h = (np.maximum(1.0 - np.abs(o - s0), 0.0)
         + np.maximum(1.0 - np.abs(o - s1), 0.0))
    b = (h * in1).astype(np.float32)
    return b, b.reshape(b.shape[0], -1).sum(axis=-1, keepdims=True)


def _quad_ref(in0, in1, s0, s1, imm2):
    b = ((np.where(in1.astype(np.float32) >= 0.0, 1.0, 0.0) * s0 - s1)
         * np.where(in0.astype(np.float32) < 0.0, 1.0, 0.0)).astype(np.float32)
    return b, None


def _ops():
    HAT = _register(
        "HAT_ACCUM",
        Bin(AluOp.MULTIPLY,
            maxx(One - Bin(AluOp.ABSOLUTE_DIFF, Src0, C0), Zero),
            Src1),
        accum=_op_add, accum_init=C1, ref=_hat_ref)
    SQAD = _register(
        "SQ_ADD",
        Bin(AluOp.ADD, Bin(AluOp.MULTIPLY, Src0, Src0),
            Bin(AluOp.MULTIPLY, Src1, Src1)),
        ref=_sqad_ref)
    QUAD = _register(
        "QUAD_FUSE",
        Bin(AluOp.MULTIPLY,
            Bin(AluOp.SUBTRACT,
                Bin(AluOp.MULTIPLY, Bin(AluOp.IS_GE, Src1, Zero), C0),
                C1),
            Bin(AluOp.IS_LT, Src0, Zero)),
        ref=_quad_ref)
    # both wrap edges in one pass: hat(o-c0)*m + hat(o-c1)*m, disjoint
    # supports so the sum is exact
    HAT2 = _register(
        "HAT2_ACCUM",
        Bin(AluOp.MULTIPLY,
            maxx(One - Bin(AluOp.MIN,
                           Bin(AluOp.ABSOLUTE_DIFF, Src0, C0),
                           Bin(AluOp.ABSOLUTE_DIFF, Src0, C1)),
                 Zero),
            Src1),
        accum=_op_add, accum_init=Zero, ref=_hat2_ref)
    return HAT, SQAD, QUAD, HAT2


# ---------- constants (match reference bit-for-bit in f32) ----------
def _gaussian_kernel2d(ksize, sigma):
    x = np.arange(ksize, dtype=np.float64) - ksize // 2
    if ksize % 2 == 0:
        x = x + 0.5
    g = np.exp(-x ** 2 / (2.0 * sigma ** 2))
    g = g / g.sum()
    return np.outer(g, g)


def _discrete_gaussian_kernel1d(ksize, sigma):
    t = float(sigma) * float(sigma)

    def iv(n, tt, terms=40):
        s = 0.0
        for k in range(terms):
            s += (tt / 2.0) ** (2 * k + n) / (math.factorial(k) * math.factorial(k + n))
        return s

    half = ksize // 2
    vals = np.array([math.exp(-t) * iv(abs(n), t) for n in range(-half, half + 1)],
                    dtype=np.float64)
    return vals / vals.sum()


def _consts():
    w8 = _gaussian_kernel2d(32, 32.0 / 6.0).astype(np.float32) / F(8.0)  # (32,32)
    w34 = np.zeros((32, 34), np.float32)
    w34[:, 0:32] = w8
    k = _discrete_gaussian_kernel1d(5, 1.6).astype(np.float32) / F(1024.0)  # (5,)
    return w34.reshape(1, 1088), [float(v) for v in k]


def _fl(t, off, n):
    """Flat element-offset view [P, n] over a multi-dim SBUF tile."""
    return bass.AP(tensor=t.tensor, offset=t.offset + off,
                   ap=[list(t.ap[0]), [1, n]])


def _build():
    HAT, SQAD, QUAD, HAT2 = _ops()
    w34_np, K = _consts()

    nc = bass.Bass(trn_type="TRN2")
    x_in = nc.declare_dram_parameter("x_in", [B_CORE, 32, 32], dt.float32,
                                     isOutput=False)
    w34_in = nc.declare_dram_parameter("w34_in", [P, 1088], dt.float32,
                                       isOutput=False)
    ang_out = nc.declare_dram_parameter("ang_out", [B_CORE, 1], dt.float32,
                                        isOutput=True)

    with tile.TileContext(nc) as tc, ExitStack() as ctx:
        cpool = ctx.enter_context(tc.tile_pool(name="c", bufs=1))
        pool = ctx.enter_context(tc.tile_pool(name="w", bufs=3))
        dma = nc.default_dma_engine
        Vx = nc.vector
        G = nc.gpsimd
        A = nc.scalar

        w34t = cpool.tile([P, 1088], dt.float32)
        dma.dma_start(out=w34t, in_=w34_in[:])
        eps_t = cpool.tile([P, 1], dt.float32)
        Vx.memset(eps_t, EPS64)
        io38 = cpool.tile([P, 38], dt.float32)
        G.iota(io38, [[1, 38]], base=0, channel_multiplier=0,
               allow_small_or_imprecise_dtypes=True)
        # persistent accumulators for the deferred tail
        extall = cpool.tile([P, N_TILES, 40], dt.float32)

        # ---- main loop: gradients + orientation + histogram only ----
        for i in range(N_TILES):
            b0 = i * P
            xp = pool.tile([P, 34, 34], dt.float32)
            dma.dma_start(out=xp[:, 1:33, 1:33], in_=x_in[b0:b0 + P])
            dma.dma_start(out=xp[:, 0:1, 1:33], in_=x_in[b0:b0 + P, 0:1, :])
            dma.dma_start(out=xp[:, 33:34, 1:33], in_=x_in[b0:b0 + P, 31:32, :])
            G.tensor_copy(xp[:, :, 0:1], xp[:, :, 1:2])
            G.tensor_copy(xp[:, :, 33:34], xp[:, :, 32:33])

            # physical slabs with lifetime-based reuse
            tA = pool.tile([P, 1122], dt.float32)  # u -> g2
            tB = pool.tile([P, 1088], dt.float32)  # S -> ra -> q18
            tC = pool.tile([P, 1088], dt.float32)  # V -> mag0 -> th
            tD = pool.tile([P, NF], dt.float32)    # gx -> tq
            tE = pool.tile([P, 1087], dt.float32)  # w -> scrV (hat out)
            gxe = pool.tile([P, NF], dt.float32)
            gy = pool.tile([P, NF], dt.float32)
            magw = pool.tile([P, NF], dt.float32)
            o_t = pool.tile([P, NF], dt.float32)

            u = tA
            S = tB
            V = tC
            gx = _fl(tD, 0, NF)
            w = tE
            G.tensor_tensor(out=u, in0=xp[:, 0:33, :], in1=xp[:, 1:34, :],
                            op=Alu.add)
            G.tensor_tensor(out=S, in0=_fl(u, 0, 1088), in1=_fl(u, 34, 1088),
                            op=Alu.add)
            G.tensor_tensor(out=V, in0=xp[:, 2:34, :], in1=xp[:, 0:32, :],
                            op=Alu.subtract)
            G.tensor_tensor(out=gx, in0=_fl(S, 2, NF), in1=_fl(S, 0, NF),
                            op=Alu.subtract)
            G.tensor_tensor(out=w, in0=_fl(V, 0, 1087), in1=_fl(V, 1, 1087),
                            op=Alu.add)
            G.tensor_tensor(out=gy, in0=_fl(w, 0, NF), in1=_fl(w, 1, NF),
                            op=Alu.add)

            # DVE: g2 (from pre-eps gx), then gxe
            g2 = _fl(tA, 0, NF)      # u dead after S
            Vx._custom_dve(SQAD, out=g2, in0=gx, in1=gy, s0=0.0, s1=0.0)
            G.tensor_scalar(out=gxe, in0=gx, scalar1=EPS8, scalar2=None,
                            op0=Alu.add)
            mag0 = _fl(tC, 0, NF)    # V dead after w
            A.activation(mag0, g2, func=Act.Sqrt, bias=eps_t, scale=1.0)

            ra = _fl(tB, 0, NF)      # S dead after gx
            scrV = _fl(tE, 0, NF)    # w dead after gy
            Vx.reciprocal_approx_accurate(out=ra, in_=gxe, scratch=scrV)
            tq = _fl(tD, 0, NF)      # gx dead after gxe+SQAD
            Vx.tensor_tensor(out=tq, in0=gy, in1=ra, op=Alu.mult)
            Vx.tensor_tensor(out=magw, in0=mag0, in1=w34t[:, 0:NF],
                             op=Alu.mult)
            th = _fl(tC, 0, NF)      # mag0 dead after magw
            A.activation(th, tq, func=Act.Arctan, bias=0.0, scale=1.0)
            q18 = _fl(tB, 0, NF)     # ra dead after tq
            Vx._custom_dve(QUAD, out=q18, in0=gxe, in1=gy, s0=36.0, s1=18.0)
            Vx.scalar_tensor_tensor(out=o_t, in0=th, scalar=C18PI, in1=q18,
                                    op0=Alu.mult, op1=Alu.add)

            base = i * 40
            Vx._custom_dve(HAT2, out=scrV, in0=o_t, in1=magw,
                           s0=-18.0, s1=18.0,
                           accum_out=_fl(extall, base + 2, 1))
            for k in range(1, 36):
                c = float(k - 18)
                Vx._custom_dve(HAT, out=scrV, in0=o_t, in1=magw,
                               s0=c, s1=0.0,
                               accum_out=_fl(extall, base + 2 + k, 1))

        # ---- deferred tail, batched across tiles (all on DVE) ----
        Vx.tensor_copy(extall[:, :, 0:2], extall[:, :, 36:38])
        Vx.tensor_copy(extall[:, :, 38:40], extall[:, :, 2:4])

        smal = cpool.tile([P, N_TILES, 36], dt.float32)
        Vx.tensor_scalar(out=smal, in0=extall[:, :, 0:36], scalar1=K[0],
                         scalar2=None, op0=Alu.mult)
        for j in range(1, 5):
            Vx.scalar_tensor_tensor(out=smal, in0=extall[:, :, j:j + 36],
                                    scalar=K[j], in1=smal,
                                    op0=Alu.mult, op1=Alu.add)

        esmall = cpool.tile([P, N_TILES, 38], dt.float32)
        Vx.tensor_copy(esmall[:, :, 1:37], smal)
        Vx.tensor_copy(esmall[:, :, 0:1], smal[:, :, 35:36])
        Vx.tensor_copy(esmall[:, :, 37:38], smal[:, :, 0:1])

        mvall = cpool.tile([P, N_TILES], dt.float32)
        idxfall = cpool.tile([P, N_TILES], dt.float32)
        ltall = cpool.tile([P, N_TILES], dt.float32)
        rtall = cpool.tile([P, N_TILES], dt.float32)
        mv = cpool.tile([P, 8], dt.float32)
        mi = cpool.tile([P, 8], dt.uint32)
        gsc = cpool.tile([P, 38], dt.float32)
        for i in range(N_TILES):
            Vx.max_with_indices(mv, mi, smal[:, i, :])
            Vx.tensor_copy(mvall[:, i:i + 1], mv[:, 0:1])
            Vx.tensor_copy(idxfall[:, i:i + 1], mi[:, 0:1])
        idx2all = cpool.tile([P, N_TILES], dt.float32)
        Vx.tensor_scalar(out=idx2all, in0=idxfall, scalar1=2.0, scalar2=None,
                         op0=Alu.add)
        for i in range(N_TILES):
            Vx.scalar_tensor_tensor(out=gsc, in0=io38,
                                    scalar=idxfall[:, i:i + 1],
                                    in1=esmall[:, i, :],
                                    op0=Alu.is_equal, op1=Alu.mult,
                                    accum_out=ltall[:, i:i + 1])
            Vx.scalar_tensor_tensor(out=gsc, in0=io38,
                                    scalar=idx2all[:, i:i + 1],
                                    in1=esmall[:, i, :],
                                    op0=Alu.is_equal, op1=Alu.mult,
                                    accum_out=rtall[:, i:i + 1])

        den0 = cpool.tile([P, N_TILES], dt.float32)
        den = cpool.tile([P, N_TILES], dt.float32)
        num = cpool.tile([P, N_TILES], dt.float32)
        Vx.tensor_tensor(out=den0, in0=ltall, in1=rtall, op=Alu.add)
        Vx.scalar_tensor_tensor(out=den, in0=mvall, scalar=-2.0,
                                in1=den0, op0=Alu.mult, op1=Alu.add)
        Vx.tensor_tensor(out=num, in0=ltall, in1=rtall, op=Alu.subtract)
        rd = cpool.tile([P, N_TILES], dt.float32)
        scr1 = cpool.tile([P, N_TILES], dt.float32)
        Vx.reciprocal_approx_accurate(out=rd, in_=den, scratch=scr1)
        nr1 = cpool.tile([P, N_TILES], dt.float32)
        Vx.tensor_tensor(out=nr1, in0=den, in1=rd, op=Alu.mult)
        Vx.tensor_scalar(out=nr1, in0=nr1, scalar1=2.0, scalar2=-1.0,
                         op0=Alu.subtract, op1=Alu.mult)  # 2 - den*rd
        Vx.tensor_tensor(out=rd, in0=nr1, in1=rd, op=Alu.mult)
        q = cpool.tile([P, N_TILES], dt.float32)
        Vx.tensor_tensor(out=q, in0=num, in1=rd, op=Alu.mult)
        angall = cpool.tile([P, N_TILES], dt.float32)
        Vx.scalar_tensor_tensor(out=q, in0=q, scalar=0.5, in1=idxfall,
                                op0=Alu.mult, op1=Alu.add)
        Vx.tensor_scalar(out=angall, in0=q, scalar1=CNPI18, scalar2=CPI,
                         op0=Alu.mult, op1=Alu.add)
        # single output DMA: ang_out[i*128 + p] = angall[p, i]
        full = ang_out[:, :]
        out_ap = bass.AP(tensor=full.tensor, offset=full.offset,
                         ap=[[1, P], [P, N_TILES]])
        dma.dma_start(out=out_ap, in_=angall)

    mybir.codegen_inst_isa_subclasses(nc)
    return nc, w34_np


def _run(patch, trace=False):
    _install_birpatch()
    patch = np.ascontiguousarray(np.asarray(patch, dtype=np.float32)
                                 .reshape(B_TOTAL, 32, 32))
    nc, w34_np = _build()
    w34_full = np.ascontiguousarray(np.broadcast_to(w34_np, (P, 1088))
                                    .astype(np.float32))
    in_maps = [
        {"x_in": np.ascontiguousarray(patch[c * B_CORE:(c + 1) * B_CORE]),
         "w34_in": w34_full}
        for c in range(N_CORES)
    ]
    kwargs = {}
    if trace:
        kwargs = dict(trace=True, trace_cores=[0])
    res = run_bass_kernel_spmd(nc, in_maps, list(range(N_CORES)), **kwargs)
    out = np.concatenate(
        [np.asarray(res.results[c]["ang_out"]).reshape(B_CORE)
         for c in range(N_CORES)])
    return out.astype(np.float32), res


def kernel(patch):
    return _run(patch)[0]


# revision 12
# speedup vs baseline: 1.2063x; 1.2063x over previous
import sys
import json
import math

sys.path.insert(0, "/opt/trn_rl_repo")

import numpy as np
from operator import add as _op_add
from contextlib import ExitStack

import concourse.bass as bass
import concourse.tile as tile
from concourse import mybir
from concourse import dve_ops
from concourse.dve_spec import (
    Spec, Bin, AluOp, Src0, Src1, C0, C1, Zero, One, maxx, lower,
)
from concourse.dve_uop import DveOpSpec
from concourse.bass_utils import run_bass_kernel_spmd

F = np.float32
PI = math.pi
dt = mybir.dt
Alu = mybir.AluOpType
Act = mybir.ActivationFunctionType

N_CORES = 8
B_TOTAL = 32768
B_CORE = B_TOTAL // N_CORES  # 4096
P = 128
N_TILES = B_CORE // P  # 32
NF = 1086  # flat width of gradient slabs ([32,34] layout, cols 32/33 junk)

EPS8 = float(F(8.0) * F(1e-8))
EPS64 = float(F(64.0) * F(1e-8))
C18PI = float(F(18.0 / PI))
CNPI18 = float(F(-PI / 18.0))
CPI = float(F(PI))


# ---------- walrus 1-wait-per-inst workaround ----------
def _split_multiwait(data):
    n = 0
    for f in data.get("functions", []):
        for blk in f.get("blocks", []):
            insts = blk.get("instructions")
            if not insts:
                continue
            out = []
            for inst in insts:
                si = inst.get("sync_info")
                waits = (si or {}).get("on_wait") or []
                if len(waits) > 1:
                    for w in waits[:-1]:
                        n += 1
                        out.append({
                            "debug": inst.get("debug", 0),
                            "engine": inst["engine"],
                            "ins": [],
                            "name": f"{inst['name']}-sw{n}",
                            "opcode": "NoOp",
                            "outs": [],
                            "sync_info": {"on_update": [], "on_wait": [w]},
                        })
                    si["on_wait"] = [waits[-1]]
                out.append(inst)
            blk["instructions"] = out
    return n


def _install_birpatch():
    if getattr(bass.Bass.to_json_bytes, "_multiwait_patched", False):
        return
    orig = bass.Bass.to_json_bytes

    def patched(self):
        data = json.loads(orig(self))
        _split_multiwait(data)
        return json.dumps(data).encode()

    patched._multiwait_patched = True
    bass.Bass.to_json_bytes = patched


# ---------- custom DVE ops ----------
def _register(name, body, accum=None, accum_init=None, ref=None):
    if name in dve_ops._SUB_OPCODE_FOR_NAME:
        return next(o for o in dve_ops.OPS if o.name == name)
    spec = Spec(body=body, accum=accum, accum_init=accum_init, reference=ref)
    row = dve_ops._CUSTOM_DVE_ROW_BASE + len(dve_ops.OPS)
    dve_ops._SUB_OPCODE_FOR_NAME[name] = row
    sha = DveOpSpec(name=name, opcode=row,
                    uops=lower(spec, ver="v3"), rd1_en=True).sha("v3")
    op = dve_ops.DveOp(name, spec, subdim=False, uops_sha={"v3": sha})
    dve_ops.OPS.append(op)
    dve_ops.CUSTOM_DVE_SPECS[name] = spec
    return op


def _hat_ref(in0, in1, s0, s1, imm2):
    b = (np.maximum(1.0 - np.abs(in0.astype(np.float32) - s0), 0.0) * in1)
    b = b.astype(np.float32)
    return b, s1 + b.reshape(b.shape[0], -1).sum(axis=-1, keepdims=True)


def _sqad_ref(in0, in1, s0, s1, imm2):
    a = in0.astype(np.float32)
    b = in1.astype(np.float32)
    return (a * a + b * b).astype(np.float32), None


def _hat2_ref(in0, in1, s0, s1, imm2):
    o = in0.astype(np.float32)
    h = (np.maximum(1.0 - np.abs(o - s0), 0.0)
         + np.maximum(1.0 - np.abs(o - s1), 0.0))
    b = (h * in1).astype(np.float32)
    return b, b.reshape(b.shape[0], -1).sum(axis=-1, keepdims=True)


def _quad_ref(in0, in1, s0, s1, imm2):
    b = ((np.where(in1.astype(np.float32) >= 0.0, 1.0, 0.0) * s0 - s1)
         * np.where(in0.astype(np.float32) < 0.0, 1.0, 0.0)).astype(np.float32)
    return b, None


def _ops():
    HAT = _register(
        "HAT_ACCUM",
        Bin(AluOp.MULTIPLY,
            maxx(One - Bin(AluOp.ABSOLUTE_DIFF, Src0, C0), Zero),
            Src1),
        accum=_op_add, accum_init=C1, ref=_hat_ref)
    SQAD = _register(
        "SQ_ADD",
        Bin(AluOp.ADD, Bin(AluOp.MULTIPLY, Src0, Src0),
            Bin(AluOp.MULTIPLY, Src1, Src1)),
        ref=_sqad_ref)
    QUAD = _register(
        "QUAD_FUSE",
        Bin(AluOp.MULTIPLY,
            Bin(AluOp.SUBTRACT,
                Bin(AluOp.MULTIPLY, Bin(AluOp.IS_GE, Src1, Zero), C0),
                C1),
            Bin(AluOp.IS_LT, Src0, Zero)),
        ref=_quad_ref)
    # both wrap edges in one pass: hat(o-c0)*m + hat(o-c1)*m, disjoint
    # supports so the sum is exact
    HAT2 = _register(
        "HAT2_ACCUM",
        Bin(AluOp.MULTIPLY,
            maxx(One - Bin(AluOp.MIN,
                           Bin(AluOp.ABSOLUTE_DIFF, Src0, C0),
                           Bin(AluOp.ABSOLUTE_DIFF, Src0, C1)),
                 Zero),
            Src1),
        accum=_op_add, accum_init=Zero, ref=_hat2_ref)
    return HAT, SQAD, QUAD, HAT2


# ---------- constants (match reference bit-for-bit in f32) ----------
def _gaussian_kernel2d(ksize, sigma):
    x = np.arange(ksize, dtype=np.float64) - ksize // 2
    if ksize % 2 == 0:
        x = x + 0.5
    g = np.exp(-x ** 2 / (2.0 * sigma ** 2))
    g = g / g.sum()
    return np.outer(g, g)


def _discrete_gaussian_kernel1d(ksize, sigma):
    t = float(sigma) * float(sigma)

    def iv(n, tt, terms=40):
        s = 0.0
        for k in range(terms):
            s += (tt / 2.0) ** (2 * k + n) / (math.factorial(k) * math.factorial(k + n))
        return s

    half = ksize // 2
    vals = np.array([math.exp(-t) * iv(abs(n), t) for n in range(-half, half + 1)],
                    dtype=np.float64)
    return vals / vals.sum()


def _consts():
    w8 = _gaussian_kernel2d(32, 32.0 / 6.0).astype(np.float32) / F(8.0)  # (32,32)
    w34 = np.zeros((32, 34), np.float32)
    w34[:, 0:32] = w8
    k = _discrete_gaussian_kernel1d(5, 1.6).astype(np.float32) / F(1024.0)  # (5,)
    return w34.reshape(1, 1088), [float(v) for v in k]


def _fl(t, off, n):
    """Flat element-offset view [P, n] over a multi-dim SBUF tile."""
    return bass.AP(tensor=t.tensor, offset=t.offset + off,
                   ap=[list(t.ap[0]), [1, n]])


def _build():
    HAT, SQAD, QUAD, HAT2 = _ops()
    w34_np, K = _consts()

    nc = bass.Bass(trn_type="TRN2")
    x_in = nc.declare_dram_parameter("x_in", [B_CORE, 32, 32], dt.float32,
                                     isOutput=False)
    w34_in = nc.declare_dram_parameter("w34_in", [P, 1088], dt.float32,
                                       isOutput=False)
    ang_out = nc.declare_dram_parameter("ang_out", [B_CORE, 1], dt.float32,
                                        isOutput=True)

    with tile.TileContext(nc) as tc, ExitStack() as ctx:
        cpool = ctx.enter_context(tc.tile_pool(name="c", bufs=1))
        pool = ctx.enter_context(tc.tile_pool(name="w", bufs=3))
        dma = nc.default_dma_engine
        Vx = nc.vector
        G = nc.gpsimd
        A = nc.scalar

        w34t = cpool.tile([P, 1088], dt.float32)
        dma.dma_start(out=w34t, in_=w34_in[:])
        eps_t = cpool.tile([P, 1], dt.float32)
        Vx.memset(eps_t, EPS64)
        io38 = cpool.tile([P, 38], dt.float32)
        G.iota(io38, [[1, 38]], base=0, channel_multiplier=0,
               allow_small_or_imprecise_dtypes=True)
        # persistent accumulators for the deferred tail
        extall = cpool.tile([P, N_TILES, 40], dt.float32)

        # ---- main loop: gradients + orientation + histogram only ----
        for i in range(N_TILES):
            b0 = i * P
            xp = pool.tile([P, 34, 34], dt.float32)
            dma.dma_start(out=xp[:, 1:33, 1:33], in_=x_in[b0:b0 + P])
            dma.dma_start(out=xp[:, 0:1, 1:33], in_=x_in[b0:b0 + P, 0:1, :])
            dma.dma_start(out=xp[:, 33:34, 1:33], in_=x_in[b0:b0 + P, 31:32, :])
            G.tensor_copy(xp[:, :, 0:1], xp[:, :, 1:2])
            G.tensor_copy(xp[:, :, 33:34], xp[:, :, 32:33])

            # physical slabs with lifetime-based reuse
            tA = pool.tile([P, 1122], dt.float32)  # u -> g2
            tB = pool.tile([P, 1088], dt.float32)  # S -> ra -> q18
            tC = pool.tile([P, 1088], dt.float32)  # V -> mag0 -> th
            tD = pool.tile([P, NF], dt.float32)    # gx -> tq
            tE = pool.tile([P, 1087], dt.float32)  # w -> scrV (hat out)
            gxe = pool.tile([P, NF], dt.float32)
            gy = pool.tile([P, NF], dt.float32)
            magw = pool.tile([P, NF], dt.float32)
            o_t = pool.tile([P, NF], dt.float32)

            u = tA
            S = tB
            V = tC
            gx = _fl(tD, 0, NF)
            w = tE
            G.tensor_tensor(out=u, in0=xp[:, 0:33, :], in1=xp[:, 1:34, :],
                            op=Alu.add)
            G.tensor_tensor(out=S, in0=_fl(u, 0, 1088), in1=_fl(u, 34, 1088),
                            op=Alu.add)
            G.tensor_tensor(out=V, in0=xp[:, 2:34, :], in1=xp[:, 0:32, :],
                            op=Alu.subtract)
            G.tensor_tensor(out=gx, in0=_fl(S, 2, NF), in1=_fl(S, 0, NF),
                            op=Alu.subtract)
            G.tensor_tensor(out=w, in0=_fl(V, 0, 1087), in1=_fl(V, 1, 1087),
                            op=Alu.add)
            G.tensor_tensor(out=gy, in0=_fl(w, 0, NF), in1=_fl(w, 1, NF),
                            op=Alu.add)

            # DVE: g2 (from pre-eps gx), then gxe
            g2 = _fl(tA, 0, NF)      # u dead after S
            Vx._custom_dve(SQAD, out=g2, in0=gx, in1=gy, s0=0.0, s1=0.0)
            Vx.tensor_scalar(out=gxe, in0=gx, scalar1=EPS8, scalar2=None,
                             op0=Alu.add)
            mag0 = _fl(tC, 0, NF)    # V dead after w
            A.activation(mag0, g2, func=Act.Sqrt, bias=eps_t, scale=1.0)

            ra = _fl(tB, 0, NF)      # S dead after gx
            scrV = _fl(tE, 0, NF)    # w dead after gy
            Vx.reciprocal_approx_accurate(out=ra, in_=gxe, scratch=scrV)
            tq = _fl(tD, 0, NF)      # gx dead after gxe+SQAD
            Vx.tensor_tensor(out=tq, in0=gy, in1=ra, op=Alu.mult)
            Vx.tensor_tensor(out=magw, in0=mag0, in1=w34t[:, 0:NF],
                             op=Alu.mult)
            th = _fl(tC, 0, NF)      # mag0 dead after magw
            A.activation(th, tq, func=Act.Arctan, bias=0.0, scale=1.0)
            q18 = _fl(tB, 0, NF)     # ra dead after tq
            Vx._custom_dve(QUAD, out=q18, in0=gxe, in1=gy, s0=36.0, s1=18.0)
            Vx.scalar_tensor_tensor(out=o_t, in0=th, scalar=C18PI, in1=q18,
                                    op0=Alu.mult, op1=Alu.add)

            base = i * 40
            Vx._custom_dve(HAT2, out=scrV, in0=o_t, in1=magw,
                           s0=-18.0, s1=18.0,
                           accum_out=_fl(extall, base + 2, 1))
            for k in range(1, 36):
                c = float(k - 18)
                Vx._custom_dve(HAT, out=scrV, in0=o_t, in1=magw,
                               s0=c, s1=0.0,
                               accum_out=_fl(extall, base + 2 + k, 1))

        # ---- deferred tail, batched across tiles (all on DVE) ----
        Vx.tensor_copy(extall[:, :, 0:2], extall[:, :, 36:38])
        Vx.tensor_copy(extall[:, :, 38:40], extall[:, :, 2:4])

        smal = cpool.tile([P, N_TILES, 36], dt.float32)
        Vx.tensor_scalar(out=smal, in0=extall[:, :, 0:36], scalar1=K[0],
                         scalar2=None, op0=Alu.mult)
        for j in range(1, 5):
            Vx.scalar_tensor_tensor(out=smal, in0=extall[:, :, j:j + 36],
                                    scalar=K[j], in1=smal,
                                    op0=Alu.mult, op1=Alu.add)

        esmall = cpool.tile([P, N_TILES, 38], dt.float32)
        Vx.tensor_copy(esmall[:, :, 1:37], smal)
        Vx.tensor_copy(esmall[:, :, 0:1], smal[:, :, 35:36])
        Vx.tensor_copy(esmall[:, :, 37:38], smal[:, :, 0:1])

        mvall = cpool.tile([P, N_TILES], dt.float32)
        idxfall = cpool.tile([P, N_TILES], dt.float32)
        ltall = cpool.tile([P, N_TILES], dt.float32)
        rtall = cpool.tile([P, N_TILES], dt.float32)
        mv = cpool.tile([P, 8], dt.float32)
        mi = cpool.tile([P, 8], dt.uint32)
        gsc = cpool.tile([P, 38], dt.float32)
        for i in range(N_TILES):
            Vx.max_with_indices(mv, mi, smal[:, i, :])
            Vx.tensor_copy(mvall[:, i:i + 1], mv[:, 0:1])
            Vx.tensor_copy(idxfall[:, i:i + 1], mi[:, 0:1])
        idx2all = cpool.tile([P, N_TILES], dt.float32)
        Vx.tensor_scalar(out=idx2all, in0=idxfall, scalar1=2.0, scalar2=None,
                         op0=Alu.add)
        for i in range(N_TILES):
            Vx.scalar_tensor_tensor(out=gsc, in0=io38,
                                    scalar=idxfall[:, i:i + 1],
                                    in1=esmall[:, i, :],
                                    op0=Alu.is_equal, op1=Alu.mult,
                                    accum_out=ltall[:, i:i + 1])
            Vx.scalar_tensor_tensor(out=gsc, in0=io38,
                                    scalar=idx2all[:, i:i + 1],
                                    in1=esmall[:, i, :],
                                    op0=Alu.is_equal, op1=Alu.mult,
                                    accum_out=rtall[:, i:i + 1])

        den0 = cpool.tile([P, N_TILES], dt.float32)
        den = cpool.tile([P, N_TILES], dt.float32)
        num = cpool.tile([P, N_TILES], dt.float32)
        Vx.tensor_tensor(out=den0, in0=ltall, in1=rtall, op=Alu.add)
        Vx.scalar_tensor_tensor(out=den, in0=mvall, scalar=-2.0,
                                in1=den0, op0=Alu.mult, op1=Alu.add)
        Vx.tensor_tensor(out=num, in0=ltall, in1=rtall, op=Alu.subtract)
        rd = cpool.tile([P, N_TILES], dt.float32)
        scr1 = cpool.tile([P, N_TILES], dt.float32)
        Vx.reciprocal_approx_accurate(out=rd, in_=den, scratch=scr1)
        nr1 = cpool.tile([P, N_TILES], dt.float32)
        Vx.tensor_tensor(out=nr1, in0=den, in1=rd, op=Alu.mult)
        Vx.tensor_scalar(out=nr1, in0=nr1, scalar1=2.0, scalar2=-1.0,
                         op0=Alu.subtract, op1=Alu.mult)  # 2 - den*rd
        Vx.tensor_tensor(out=rd, in0=nr1, in1=rd, op=Alu.mult)
        q = cpool.tile([P, N_TILES], dt.float32)
        Vx.tensor_tensor(out=q, in0=num, in1=rd, op=Alu.mult)
        angall = cpool.tile([P, N_TILES], dt.float32)
        Vx.scalar_tensor_tensor(out=q, in0=q, scalar=0.5, in1=idxfall,
                                op0=Alu.mult, op1=Alu.add)
        Vx.tensor_scalar(out=angall, in0=q, scalar1=CNPI18, scalar2=CPI,
                         op0=Alu.mult, op1=Alu.add)
        # single output DMA: ang_out[i*128 + p] = angall[p, i]
        full = ang_out[:, :]
        out_ap = bass.AP(tensor=full.tensor, offset=full.offset,
                         ap=[[1, P], [P, N_TILES]])
        dma.dma_start(out=out_ap, in_=angall)

    mybir.codegen_inst_isa_subclasses(nc)
    return nc, w34_np


def _run(patch, trace=False):
    _install_birpatch()
    patch = np.ascontiguousarray(np.asarray(patch, dtype=np.float32)
                                 .reshape(B_TOTAL, 32, 32))
    nc, w34_np = _build()
    w34_full = np.ascontiguousarray(np.broadcast_to(w34_np, (P, 1088))
                                    .astype(np.float32))
    in_maps = [
        {"x_in": np.ascontiguousarray(patch[c * B_CORE:(c + 1) * B_CORE]),
         "w34_in": w34_full}
        for c in range(N_CORES)
    ]
    kwargs = {}
    if trace:
        kwargs = dict(trace=True, trace_cores=[0])
    res = run_bass_kernel_spmd(nc, in_maps, list(range(N_CORES)), **kwargs)
    out = np.concatenate(
        [np.asarray(res.results[c]["ang_out"]).reshape(B_CORE)
         for c in range(N_CORES)])
    return out.astype(np.float32), res


def kernel(patch):
    return _run(patch)[0]


# revision 15
# speedup vs baseline: 1.2382x; 1.0265x over previous
import sys
import json
import math

sys.path.insert(0, "/opt/trn_rl_repo")

import numpy as np
from operator import add as _op_add
from contextlib import ExitStack

import concourse.bass as bass
import concourse.tile as tile
from concourse import mybir
from concourse import dve_ops
from concourse.dve_spec import (
    Spec, Bin, AluOp, Src0, Src1, C0, C1, Zero, One, maxx, lower,
)
from concourse.dve_uop import DveOpSpec
from concourse.bass_utils import run_bass_kernel_spmd

F = np.float32
PI = math.pi
dt = mybir.dt
Alu = mybir.AluOpType
Act = mybir.ActivationFunctionType

N_CORES = 8
B_TOTAL = 32768
B_CORE = B_TOTAL // N_CORES  # 4096
P = 128
N_TILES = B_CORE // P  # 32
NF = 1086  # flat width of gradient slabs ([32,34] layout, cols 32/33 junk)

EPS8 = float(F(8.0) * F(1e-8))
EPS64 = float(F(64.0) * F(1e-8))
C18PI = float(F(18.0 / PI))
CNPI18 = float(F(-PI / 18.0))
CPI = float(F(PI))


# ---------- walrus 1-wait-per-inst workaround ----------
def _split_multiwait(data):
    n = 0
    for f in data.get("functions", []):
        for blk in f.get("blocks", []):
            insts = blk.get("instructions")
            if not insts:
                continue
            out = []
            for inst in insts:
                si = inst.get("sync_info")
                waits = (si or {}).get("on_wait") or []
                if len(waits) > 1:
                    for w in waits[:-1]:
                        n += 1
                        out.append({
                            "debug": inst.get("debug", 0),
                            "engine": inst["engine"],
                            "ins": [],
                            "name": f"{inst['name']}-sw{n}",
                            "opcode": "NoOp",
                            "outs": [],
                            "sync_info": {"on_update": [], "on_wait": [w]},
                        })
                    si["on_wait"] = [waits[-1]]
                out.append(inst)
            blk["instructions"] = out
    return n


def _install_birpatch():
    if getattr(bass.Bass.to_json_bytes, "_multiwait_patched", False):
        return
    orig = bass.Bass.to_json_bytes

    def patched(self):
        data = json.loads(orig(self))
        _split_multiwait(data)
        return json.dumps(data).encode()

    patched._multiwait_patched = True
    bass.Bass.to_json_bytes = patched


# ---------- custom DVE ops ----------
def _register(name, body, accum=None, accum_init=None, ref=None):
    if name in dve_ops._SUB_OPCODE_FOR_NAME:
        return next(o for o in dve_ops.OPS if o.name == name)
    spec = Spec(body=body, accum=accum, accum_init=accum_init, reference=ref)
    row = dve_ops._CUSTOM_DVE_ROW_BASE + len(dve_ops.OPS)
    dve_ops._SUB_OPCODE_FOR_NAME[name] = row
    sha = DveOpSpec(name=name, opcode=row,
                    uops=lower(spec, ver="v3"), rd1_en=True).sha("v3")
    op = dve_ops.DveOp(name, spec, subdim=False, uops_sha={"v3": sha})
    dve_ops.OPS.append(op)
    dve_ops.CUSTOM_DVE_SPECS[name] = spec
    return op


def _hat_ref(in0, in1, s0, s1, imm2):
    b = (np.maximum(1.0 - np.abs(in0.astype(np.float32) - s0), 0.0) * in1)
    b = b.astype(np.float32)
    return b, s1 + b.reshape(b.shape[0], -1).sum(axis=-1, keepdims=True)


def _sqad_ref(in0, in1, s0, s1, imm2):
    a = in0.astype(np.float32)
    b = in1.astype(np.float32)
    return (a * a + b * b).astype(np.float32), None


def _hat2_ref(in0, in1, s0, s1, imm2):
    o = in0.astype(np.float32)
    h = (np.maximum(1.0 - np.abs(o - s0), 0.0)
         + np.maximum(1.0 - np.abs(o - s1), 0.0))
    b = (h * in1).astype(np.float32)
    return b, b.reshape(b.shape[0], -1).sum(axis=-1, keepdims=True)


def _quad_ref(in0, in1, s0, s1, imm2):
    b = ((np.where(in1.astype(np.float32) >= 0.0, 1.0, 0.0) * s0 - s1)
         * np.where(in0.astype(np.float32) < 0.0, 1.0, 0.0)).astype(np.float32)
    return b, None


def _ops():
    HAT = _register(
        "HAT_ACCUM",
        Bin(AluOp.MULTIPLY,
            maxx(One - Bin(AluOp.ABSOLUTE_DIFF, Src0, C0), Zero),
            Src1),
        accum=_op_add, accum_init=C1, ref=_hat_ref)
    SQAD = _register(
        "SQ_ADD",
        Bin(AluOp.ADD, Bin(AluOp.MULTIPLY, Src0, Src0),
            Bin(AluOp.MULTIPLY, Src1, Src1)),
        ref=_sqad_ref)
    QUAD = _register(
        "QUAD_FUSE",
        Bin(AluOp.MULTIPLY,
            Bin(AluOp.SUBTRACT,
                Bin(AluOp.MULTIPLY, Bin(AluOp.IS_GE, Src1, Zero), C0),
                C1),
            Bin(AluOp.IS_LT, Src0, Zero)),
        ref=_quad_ref)
    # both wrap edges in one pass: hat(o-c0)*m + hat(o-c1)*m, disjoint
    # supports so the sum is exact
    HAT2 = _register(
        "HAT2_ACCUM",
        Bin(AluOp.MULTIPLY,
            maxx(One - Bin(AluOp.MIN,
                           Bin(AluOp.ABSOLUTE_DIFF, Src0, C0),
                           Bin(AluOp.ABSOLUTE_DIFF, Src0, C1)),
                 Zero),
            Src1),
        accum=_op_add, accum_init=Zero, ref=_hat2_ref)
    return HAT, SQAD, QUAD, HAT2


# ---------- constants (match reference bit-for-bit in f32) ----------
def _gaussian_kernel2d(ksize, sigma):
    x = np.arange(ksize, dtype=np.float64) - ksize // 2
    if ksize % 2 == 0:
        x = x + 0.5
    g = np.exp(-x ** 2 / (2.0 * sigma ** 2))
    g = g / g.sum()
    return np.outer(g, g)


def _discrete_gaussian_kernel1d(ksize, sigma):
    t = float(sigma) * float(sigma)

    def iv(n, tt, terms=40):
        s = 0.0
        for k in range(terms):
            s += (tt / 2.0) ** (2 * k + n) / (math.factorial(k) * math.factorial(k + n))
        return s

    half = ksize // 2
    vals = np.array([math.exp(-t) * iv(abs(n), t) for n in range(-half, half + 1)],
                    dtype=np.float64)
    return vals / vals.sum()


def _consts():
    w8 = _gaussian_kernel2d(32, 32.0 / 6.0).astype(np.float32) / F(8.0)  # (32,32)
    k = _discrete_gaussian_kernel1d(5, 1.6).astype(np.float32) / F(1024.0)  # (5,)
    return w8.reshape(1, 1024), [float(v) for v in k]


def _g32(t):
    """[P, 32, 32] view over a [P, >=1086] flat slab laid out as 32x34 rows."""
    return bass.AP(tensor=t.tensor, offset=t.offset,
                   ap=[list(t.ap[0]), [34, 32], [1, 32]])


def _fl(t, off, n):
    """Flat element-offset view [P, n] over a multi-dim SBUF tile."""
    return bass.AP(tensor=t.tensor, offset=t.offset + off,
                   ap=[list(t.ap[0]), [1, n]])


def _build():
    HAT, SQAD, QUAD, HAT2 = _ops()
    w34_np, K = _consts()

    nc = bass.Bass(trn_type="TRN2")
    x_in = nc.declare_dram_parameter("x_in", [B_CORE, 32, 32], dt.float32,
                                     isOutput=False)
    w34_in = nc.declare_dram_parameter("w34_in", [P, 1024], dt.float32,
                                       isOutput=False)
    ang_out = nc.declare_dram_parameter("ang_out", [B_CORE, 1], dt.float32,
                                        isOutput=True)

    with tile.TileContext(nc) as tc, ExitStack() as ctx:
        cpool = ctx.enter_context(tc.tile_pool(name="c", bufs=1))
        pool = ctx.enter_context(tc.tile_pool(name="w", bufs=3))
        dma = nc.default_dma_engine
        Vx = nc.vector
        G = nc.gpsimd
        A = nc.scalar

        w34t = cpool.tile([P, 1024], dt.float32)
        dma.dma_start(out=w34t, in_=w34_in[:])
        eps_t = cpool.tile([P, 1], dt.float32)
        Vx.memset(eps_t, EPS64)
        io38 = cpool.tile([P, 38], dt.float32)
        G.iota(io38, [[1, 38]], base=0, channel_multiplier=0,
               allow_small_or_imprecise_dtypes=True)
        # persistent accumulators for the deferred tail
        extall = cpool.tile([P, N_TILES, 40], dt.float32)

        # ---- main loop: gradients + orientation + histogram only ----
        for i in range(N_TILES):
            b0 = i * P
            xp = pool.tile([P, 34, 34], dt.float32)
            dma.dma_start(out=xp[:, 1:33, 1:33], in_=x_in[b0:b0 + P])
            dma.dma_start(out=xp[:, 0:1, 1:33], in_=x_in[b0:b0 + P, 0:1, :])
            dma.dma_start(out=xp[:, 33:34, 1:33], in_=x_in[b0:b0 + P, 31:32, :])
            G.tensor_copy(xp[:, :, 0:1], xp[:, :, 1:2])
            G.tensor_copy(xp[:, :, 33:34], xp[:, :, 32:33])

            # physical slabs with lifetime-based reuse
            tA = pool.tile([P, 1122], dt.float32)  # u -> g2
            tB = pool.tile([P, 1088], dt.float32)  # S -> ra -> q18
            tC = pool.tile([P, 1088], dt.float32)  # V -> mag0 -> th
            tD = pool.tile([P, NF], dt.float32)    # gx -> tq
            tE = pool.tile([P, 1087], dt.float32)  # w -> scrV (hat out)
            gxe = pool.tile([P, NF], dt.float32)
            gy = pool.tile([P, NF], dt.float32)
            magw = pool.tile([P, 1024], dt.float32)
            o_t = pool.tile([P, 1024], dt.float32)

            u = tA
            S = tB
            V = tC
            gx = _fl(tD, 0, NF)
            w = tE
            G.tensor_tensor(out=u, in0=xp[:, 0:33, :], in1=xp[:, 1:34, :],
                            op=Alu.add)
            G.tensor_tensor(out=S, in0=_fl(u, 0, 1088), in1=_fl(u, 34, 1088),
                            op=Alu.add)
            G.tensor_tensor(out=V, in0=xp[:, 2:34, :], in1=xp[:, 0:32, :],
                            op=Alu.subtract)
            G.tensor_tensor(out=gx, in0=_fl(S, 2, NF), in1=_fl(S, 0, NF),
                            op=Alu.subtract)
            G.tensor_tensor(out=w, in0=_fl(V, 0, 1087), in1=_fl(V, 1, 1087),
                            op=Alu.add)
            G.tensor_tensor(out=gy, in0=_fl(w, 0, NF), in1=_fl(w, 1, NF),
                            op=Alu.add)

            # DVE: g2 (from pre-eps gx), then gxe
            g2 = _fl(tA, 0, NF)      # u dead after S
            Vx._custom_dve(SQAD, out=g2, in0=gx, in1=gy, s0=0.0, s1=0.0)
            Vx.tensor_scalar(out=gxe, in0=gx, scalar1=EPS8, scalar2=None,
                             op0=Alu.add)
            mag0 = _fl(tC, 0, NF)    # V dead after w
            A.activation(mag0, g2, func=Act.Sqrt, bias=eps_t, scale=1.0)

            ra = _fl(tB, 0, NF)      # S dead after gx
            scrV = _fl(tE, 0, NF)    # w dead after gy
            Vx.reciprocal_approx_accurate(out=ra, in_=gxe, scratch=scrV)
            tq = _fl(tD, 0, NF)      # gx dead after gxe+SQAD
            Vx.tensor_tensor(out=tq, in0=gy, in1=ra, op=Alu.mult)
            Vx.tensor_tensor(out=magw, in0=_g32(tC), in1=w34t,
                             op=Alu.mult)
            th = _fl(tC, 0, NF)      # mag0 dead after magw
            A.activation(th, tq, func=Act.Arctan, bias=0.0, scale=1.0)
            q18 = _fl(tB, 0, NF)     # ra dead after tq
            Vx._custom_dve(QUAD, out=q18, in0=gxe, in1=gy, s0=36.0, s1=18.0)
            o_f = _fl(tD, 0, NF)     # tq dead after arctan
            Vx.scalar_tensor_tensor(out=o_f, in0=th, scalar=C18PI, in1=q18,
                                    op0=Alu.mult, op1=Alu.add)
            Vx.tensor_scalar(out=o_t, in0=_g32(tD), scalar1=1.0,
                             scalar2=None, op0=Alu.mult)

            base = i * 40
            scrH = _fl(tE, 0, 1024)
            Vx._custom_dve(HAT2, out=scrH, in0=o_t, in1=magw,
                           s0=-18.0, s1=18.0,
                           accum_out=_fl(extall, base + 2, 1))
            for k in range(1, 36):
                c = float(k - 18)
                Vx._custom_dve(HAT, out=scrH, in0=o_t, in1=magw,
                               s0=c, s1=0.0,
                               accum_out=_fl(extall, base + 2 + k, 1))

        # ---- deferred tail, batched across tiles (all on DVE) ----
        Vx.tensor_copy(extall[:, :, 0:2], extall[:, :, 36:38])
        Vx.tensor_copy(extall[:, :, 38:40], extall[:, :, 2:4])

        smal = cpool.tile([P, N_TILES, 36], dt.float32)
        Vx.tensor_scalar(out=smal, in0=extall[:, :, 0:36], scalar1=K[0],
                         scalar2=None, op0=Alu.mult)
        for j in range(1, 5):
            Vx.scalar_tensor_tensor(out=smal, in0=extall[:, :, j:j + 36],
                                    scalar=K[j], in1=smal,
                                    op0=Alu.mult, op1=Alu.add)

        esmall = cpool.tile([P, N_TILES, 38], dt.float32)
        Vx.tensor_copy(esmall[:, :, 1:37], smal)
        Vx.tensor_copy(esmall[:, :, 0:1], smal[:, :, 35:36])
        Vx.tensor_copy(esmall[:, :, 37:38], smal[:, :, 0:1])

        mvall = cpool.tile([P, N_TILES], dt.float32)
        idxfall = cpool.tile([P, N_TILES], dt.float32)
        ltall = cpool.tile([P, N_TILES], dt.float32)
        rtall = cpool.tile([P, N_TILES], dt.float32)
        mv = cpool.tile([P, 8], dt.float32)
        mi = cpool.tile([P, 8], dt.uint32)
        gsc = cpool.tile([P, 38], dt.float32)
        for i in range(N_TILES):
            Vx.max_with_indices(mv, mi, smal[:, i, :])
            Vx.tensor_copy(mvall[:, i:i + 1], mv[:, 0:1])
            Vx.tensor_copy(idxfall[:, i:i + 1], mi[:, 0:1])
        idx2all = cpool.tile([P, N_TILES], dt.float32)
        Vx.tensor_scalar(out=idx2all, in0=idxfall, scalar1=2.0, scalar2=None,
                         op0=Alu.add)
        for i in range(N_TILES):
            Vx.scalar_tensor_tensor(out=gsc, in0=io38,
                                    scalar=idxfall[:, i:i + 1],
                                    in1=esmall[:, i, :],
                                    op0=Alu.is_equal, op1=Alu.mult,
                                    accum_out=ltall[:, i:i + 1])
            Vx.scalar_tensor_tensor(out=gsc, in0=io38,
                                    scalar=idx2all[:, i:i + 1],
                                    in1=esmall[:, i, :],
                                    op0=Alu.is_equal, op1=Alu.mult,
                                    accum_out=rtall[:, i:i + 1])

        den0 = cpool.tile([P, N_TILES], dt.float32)
        den = cpool.tile([P, N_TILES], dt.float32)
        num = cpool.tile([P, N_TILES], dt.float32)
        Vx.tensor_tensor(out=den0, in0=ltall, in1=rtall, op=Alu.add)
        Vx.scalar_tensor_tensor(out=den, in0=mvall, scalar=-2.0,
                                in1=den0, op0=Alu.mult, op1=Alu.add)
        Vx.tensor_tensor(out=num, in0=ltall, in1=rtall, op=Alu.subtract)
        rd = cpool.tile([P, N_TILES], dt.float32)
        scr1 = cpool.tile([P, N_TILES], dt.float32)
        Vx.reciprocal_approx_accurate(out=rd, in_=den, scratch=scr1)
        nr1 = cpool.tile([P, N_TILES], dt.float32)
        Vx.tensor_tensor(out=nr1, in0=den, in1=rd, op=Alu.mult)
        Vx.tensor_scalar(out=nr1, in0=nr1, scalar1=2.0, scalar2=-1.0,
                         op0=Alu.subtract, op1=Alu.mult)  # 2 - den*rd
        Vx.tensor_tensor(out=rd, in0=nr1, in1=rd, op=Alu.mult)
        q = cpool.tile([P, N_TILES], dt.float32)
        Vx.tensor_tensor(out=q, in0=num, in1=rd, op=Alu.mult)
        angall = cpool.tile([P, N_TILES], dt.float32)
        Vx.scalar_tensor_tensor(out=q, in0=q, scalar=0.5, in1=idxfall,
                                op0=Alu.mult, op1=Alu.add)
        Vx.tensor_scalar(out=angall, in0=q, scalar1=CNPI18, scalar2=CPI,
                         op0=Alu.mult, op1=Alu.add)
        # single output DMA: ang_out[i*128 + p] = angall[p, i]
        full = ang_out[:, :]
        out_ap = bass.AP(tensor=full.tensor, offset=full.offset,
                         ap=[[1, P], [P, N_TILES]])
        dma.dma_start(out=out_ap, in_=angall)

    mybir.codegen_inst_isa_subclasses(nc)
    return nc, w34_np


def _run(patch, trace=False):
    _install_birpatch()
    patch = np.ascontiguousarray(np.asarray(patch, dtype=np.float32)
                                 .reshape(B_TOTAL, 32, 32))
    nc, w34_np = _build()
    w34_full = np.ascontiguousarray(np.broadcast_to(w34_np, (P, 1024))
                                    .astype(np.float32))
    in_maps = [
        {"x_in": np.ascontiguousarray(patch[c * B_CORE:(c + 1) * B_CORE]),
         "w34_in": w34_full}
        for c in range(N_CORES)
    ]
    kwargs = {}
    if trace:
        kwargs = dict(trace=True, trace_cores=[0])
    res = run_bass_kernel_spmd(nc, in_maps, list(range(N_CORES)), **kwargs)
    out = np.concatenate(
        [np.asarray(res.results[c]["ang_out"]).reshape(B_CORE)
         for c in range(N_CORES)])
    return out.astype(np.float32), res


def kernel(patch):
    return _run(patch)[0]


# revision 17
# speedup vs baseline: 1.2388x; 1.0004x over previous
import sys
import json
import math

sys.path.insert(0, "/opt/trn_rl_repo")

import numpy as np
from operator import add as _op_add
from contextlib import ExitStack

import concourse.bass as bass
import concourse.tile as tile
from concourse import mybir
from concourse import dve_ops
from concourse.dve_spec import (
    Spec, Bin, AluOp, Src0, Src1, C0, C1, Zero, One, maxx, lower,
)
from concourse.dve_uop import DveOpSpec
from concourse.bass_utils import run_bass_kernel_spmd

F = np.float32
PI = math.pi
dt = mybir.dt
Alu = mybir.AluOpType
Act = mybir.ActivationFunctionType

N_CORES = 8
B_TOTAL = 32768
B_CORE = B_TOTAL // N_CORES  # 4096
P = 128
N_TILES = B_CORE // P  # 32
NF = 1086  # flat width of gradient slabs ([32,34] layout, cols 32/33 junk)

EPS8 = float(F(8.0) * F(1e-8))
EPS64 = float(F(64.0) * F(1e-8))
C18PI = float(F(18.0 / PI))
CNPI18 = float(F(-PI / 18.0))
CPI = float(F(PI))


# ---------- walrus 1-wait-per-inst workaround ----------
def _split_multiwait(data):
    n = 0
    for f in data.get("functions", []):
        for blk in f.get("blocks", []):
            insts = blk.get("instructions")
            if not insts:
                continue
            out = []
            for inst in insts:
                si = inst.get("sync_info")
                waits = (si or {}).get("on_wait") or []
                if len(waits) > 1:
                    for w in waits[:-1]:
                        n += 1
                        out.append({
                            "debug": inst.get("debug", 0),
                            "engine": inst["engine"],
                            "ins": [],
                            "name": f"{inst['name']}-sw{n}",
                            "opcode": "NoOp",
                            "outs": [],
                            "sync_info": {"on_update": [], "on_wait": [w]},
                        })
                    si["on_wait"] = [waits[-1]]
                out.append(inst)
            blk["instructions"] = out
    return n


def _install_birpatch():
    if getattr(bass.Bass.to_json_bytes, "_multiwait_patched", False):
        return
    orig = bass.Bass.to_json_bytes

    def patched(self):
        data = json.loads(orig(self))
        _split_multiwait(data)
        return json.dumps(data).encode()

    patched._multiwait_patched = True
    bass.Bass.to_json_bytes = patched


# ---------- custom DVE ops ----------
def _register(name, body, accum=None, accum_init=None, ref=None):
    if name in dve_ops._SUB_OPCODE_FOR_NAME:
        return next(o for o in dve_ops.OPS if o.name == name)
    spec = Spec(body=body, accum=accum, accum_init=accum_init, reference=ref)
    row = dve_ops._CUSTOM_DVE_ROW_BASE + len(dve_ops.OPS)
    dve_ops._SUB_OPCODE_FOR_NAME[name] = row
    sha = DveOpSpec(name=name, opcode=row,
                    uops=lower(spec, ver="v3"), rd1_en=True).sha("v3")
    op = dve_ops.DveOp(name, spec, subdim=False, uops_sha={"v3": sha})
    dve_ops.OPS.append(op)
    dve_ops.CUSTOM_DVE_SPECS[name] = spec
    return op


def _hat_ref(in0, in1, s0, s1, imm2):
    b = (np.maximum(1.0 - np.abs(in0.astype(np.float32) - s0), 0.0) * in1)
    b = b.astype(np.float32)
    return b, s1 + b.reshape(b.shape[0], -1).sum(axis=-1, keepdims=True)


def _sqad_ref(in0, in1, s0, s1, imm2):
    a = in0.astype(np.float32)
    b = in1.astype(np.float32)
    return (a * a + b * b).astype(np.float32), None


def _hat2_ref(in0, in1, s0, s1, imm2):
    o = in0.astype(np.float32)
    h = (np.maximum(1.0 - np.abs(o - s0), 0.0)
         + np.maximum(1.0 - np.abs(o - s1), 0.0))
    b = (h * in1).astype(np.float32)
    return b, b.reshape(b.shape[0], -1).sum(axis=-1, keepdims=True)


def _quad_ref(in0, in1, s0, s1, imm2):
    b = ((np.where(in1.astype(np.float32) >= 0.0, 1.0, 0.0) * s0 - s1)
         * np.where(in0.astype(np.float32) < 0.0, 1.0, 0.0)).astype(np.float32)
    return b, None


def _ops():
    HAT = _register(
        "HAT_ACCUM",
        Bin(AluOp.MULTIPLY,
            maxx(One - Bin(AluOp.ABSOLUTE_DIFF, Src0, C0), Zero),
            Src1),
        accum=_op_add, accum_init=C1, ref=_hat_ref)
    SQAD = _register(
        "SQ_ADD",
        Bin(AluOp.ADD, Bin(AluOp.MULTIPLY, Src0, Src0),
            Bin(AluOp.MULTIPLY, Src1, Src1)),
        ref=_sqad_ref)
    QUAD = _register(
        "QUAD_FUSE",
        Bin(AluOp.MULTIPLY,
            Bin(AluOp.SUBTRACT,
                Bin(AluOp.MULTIPLY, Bin(AluOp.IS_GE, Src1, Zero), C0),
                C1),
            Bin(AluOp.IS_LT, Src0, Zero)),
        ref=_quad_ref)
    # both wrap edges in one pass: hat(o-c0)*m + hat(o-c1)*m, disjoint
    # supports so the sum is exact
    HAT2 = _register(
        "HAT2_ACCUM",
        Bin(AluOp.MULTIPLY,
            maxx(One - Bin(AluOp.MIN,
                           Bin(AluOp.ABSOLUTE_DIFF, Src0, C0),
                           Bin(AluOp.ABSOLUTE_DIFF, Src0, C1)),
                 Zero),
            Src1),
        accum=_op_add, accum_init=Zero, ref=_hat2_ref)
    return HAT, SQAD, QUAD, HAT2


# ---------- constants (match reference bit-for-bit in f32) ----------
def _gaussian_kernel2d(ksize, sigma):
    x = np.arange(ksize, dtype=np.float64) - ksize // 2
    if ksize % 2 == 0:
        x = x + 0.5
    g = np.exp(-x ** 2 / (2.0 * sigma ** 2))
    g = g / g.sum()
    return np.outer(g, g)


def _discrete_gaussian_kernel1d(ksize, sigma):
    t = float(sigma) * float(sigma)

    def iv(n, tt, terms=40):
        s = 0.0
        for k in range(terms):
            s += (tt / 2.0) ** (2 * k + n) / (math.factorial(k) * math.factorial(k + n))
        return s

    half = ksize // 2
    vals = np.array([math.exp(-t) * iv(abs(n), t) for n in range(-half, half + 1)],
                    dtype=np.float64)
    return vals / vals.sum()


def _consts():
    w8 = _gaussian_kernel2d(32, 32.0 / 6.0).astype(np.float32) / F(8.0)  # (32,32)
    k = _discrete_gaussian_kernel1d(5, 1.6).astype(np.float32) / F(1024.0)  # (5,)
    return w8.reshape(1, 1024), [float(v) for v in k]


def _g34(t, off=0):
    """[P, 32, 32] strided view (32x34-row layout) over a flat slab."""
    return bass.AP(tensor=t.tensor, offset=t.offset + off,
                   ap=[list(t.ap[0]), [34, 32], [1, 32]])


def _fl(t, off, n):
    """Flat element-offset view [P, n] over a multi-dim SBUF tile."""
    return bass.AP(tensor=t.tensor, offset=t.offset + off,
                   ap=[list(t.ap[0]), [1, n]])


def _build():
    HAT, SQAD, QUAD, HAT2 = _ops()
    w34_np, K = _consts()

    nc = bass.Bass(trn_type="TRN2")
    x_in = nc.declare_dram_parameter("x_in", [B_CORE, 32, 32], dt.float32,
                                     isOutput=False)
    w34_in = nc.declare_dram_parameter("w34_in", [P, 1024], dt.float32,
                                       isOutput=False)
    ang_out = nc.declare_dram_parameter("ang_out", [B_CORE, 1], dt.float32,
                                        isOutput=True)

    with tile.TileContext(nc) as tc, ExitStack() as ctx:
        cpool = ctx.enter_context(tc.tile_pool(name="c", bufs=1))
        pool = ctx.enter_context(tc.tile_pool(name="w", bufs=4))
        dma = nc.default_dma_engine
        Vx = nc.vector
        G = nc.gpsimd
        A = nc.scalar

        w34t = cpool.tile([P, 1024], dt.float32)
        dma.dma_start(out=w34t, in_=w34_in[:])
        eps_t = cpool.tile([P, 1], dt.float32)
        Vx.memset(eps_t, EPS64)
        io38 = cpool.tile([P, 38], dt.float32)
        G.iota(io38, [[1, 38]], base=0, channel_multiplier=0,
               allow_small_or_imprecise_dtypes=True)
        # persistent accumulators for the deferred tail
        extall = cpool.tile([P, N_TILES, 40], dt.float32)

        # ---- main loop: gradients + orientation + histogram only ----
        for i in range(N_TILES):
            b0 = i * P
            xp = pool.tile([P, 34, 34], dt.float32)
            dma.dma_start(out=xp[:, 1:33, 1:33], in_=x_in[b0:b0 + P])
            dma.dma_start(out=xp[:, 0:1, 1:33], in_=x_in[b0:b0 + P, 0:1, :])
            dma.dma_start(out=xp[:, 33:34, 1:33], in_=x_in[b0:b0 + P, 31:32, :])
            G.tensor_copy(xp[:, :, 0:1], xp[:, :, 1:2])
            G.tensor_copy(xp[:, :, 33:34], xp[:, :, 32:33])

            # physical slabs with lifetime-based reuse
            tA = pool.tile([P, 1122], dt.float32)  # u -> g2
            tB = pool.tile([P, 1088], dt.float32)  # S -> ra -> q18
            tC = pool.tile([P, 1088], dt.float32)  # V -> mag0 -> th
            tD = pool.tile([P, NF], dt.float32)    # gx -> tq -> o_f
            tE = pool.tile([P, 1087], dt.float32)  # w -> hat scratch
            gxe = pool.tile([P, NF], dt.float32)
            gy = pool.tile([P, NF], dt.float32)
            magw = pool.tile([P, 1024], dt.float32)
            o_t = pool.tile([P, 1024], dt.float32)

            u = tA
            S = tB
            V = tC
            gx = _fl(tD, 0, NF)
            w = tE
            G.tensor_tensor(out=u, in0=xp[:, 0:33, :], in1=xp[:, 1:34, :],
                            op=Alu.add)
            G.tensor_tensor(out=S, in0=_fl(u, 0, 1088), in1=_fl(u, 34, 1088),
                            op=Alu.add)
            G.tensor_tensor(out=V, in0=xp[:, 2:34, :], in1=xp[:, 0:32, :],
                            op=Alu.subtract)
            G.tensor_tensor(out=gx, in0=_fl(S, 2, NF), in1=_fl(S, 0, NF),
                            op=Alu.subtract)
            G.tensor_tensor(out=w, in0=_fl(V, 0, 1087), in1=_fl(V, 1, 1087),
                            op=Alu.add)
            G.tensor_tensor(out=gy, in0=_fl(w, 0, NF), in1=_fl(w, 1, NF),
                            op=Alu.add)

            # DVE: g2 (from pre-eps gx), then gxe
            g2 = _fl(tA, 0, NF)      # u dead after S
            Vx._custom_dve(SQAD, out=g2, in0=gx, in1=gy, s0=0.0, s1=0.0)
            Vx.tensor_scalar(out=gxe, in0=gx, scalar1=EPS8, scalar2=None,
                             op0=Alu.add)
            mag0 = _fl(tC, 0, NF)    # V dead after w
            A.activation(mag0, g2, func=Act.Sqrt, bias=eps_t, scale=1.0)

            ra = _fl(tB, 0, NF)      # S dead after gx
            scrV = _fl(tE, 0, NF)    # w dead after gy
            Vx.reciprocal_approx_accurate(out=ra, in_=gxe, scratch=scrV)
            tq = _fl(tD, 0, NF)      # gx dead after gxe+SQAD
            Vx.tensor_tensor(out=tq, in0=gy, in1=ra, op=Alu.mult)
            Vx.tensor_tensor(out=magw, in0=_g34(tC), in1=w34t,
                             op=Alu.mult)
            th = _fl(tC, 0, NF)      # mag0 dead after magw
            A.activation(th, tq, func=Act.Arctan, bias=0.0, scale=1.0)
            q18 = _fl(tB, 0, NF)     # ra dead after tq
            Vx._custom_dve(QUAD, out=q18, in0=gxe, in1=gy, s0=36.0, s1=18.0)
            o_f = _fl(tD, 0, NF)     # tq dead after arctan
            Vx.scalar_tensor_tensor(out=o_f, in0=th, scalar=C18PI, in1=q18,
                                    op0=Alu.mult, op1=Alu.add)
            Vx.tensor_scalar(out=o_t, in0=_g34(tD), scalar1=1.0,
                             scalar2=None, op0=Alu.mult)

            base = i * 40
            scrH = _fl(tE, 0, 1024)
            Vx._custom_dve(HAT2, out=scrH, in0=o_t, in1=magw,
                           s0=-18.0, s1=18.0,
                           accum_out=_fl(extall, base + 2, 1))
            for k in range(1, 36):
                c = float(k - 18)
                Vx._custom_dve(HAT, out=scrH, in0=o_t, in1=magw,
                               s0=c, s1=0.0,
                               accum_out=_fl(extall, base + 2 + k, 1))

        # ---- deferred tail, batched across tiles (all on DVE) ----
        Vx.tensor_copy(extall[:, :, 0:2], extall[:, :, 36:38])
        Vx.tensor_copy(extall[:, :, 38:40], extall[:, :, 2:4])

        smal = cpool.tile([P, N_TILES, 36], dt.float32)
        Vx.tensor_scalar(out=smal, in0=extall[:, :, 0:36], scalar1=K[0],
                         scalar2=None, op0=Alu.mult)
        for j in range(1, 5):
            Vx.scalar_tensor_tensor(out=smal, in0=extall[:, :, j:j + 36],
                                    scalar=K[j], in1=smal,
                                    op0=Alu.mult, op1=Alu.add)

        esmall = cpool.tile([P, N_TILES, 38], dt.float32)
        Vx.tensor_copy(esmall[:, :, 1:37], smal)
        Vx.tensor_copy(esmall[:, :, 0:1], smal[:, :, 35:36])
        Vx.tensor_copy(esmall[:, :, 37:38], smal[:, :, 0:1])

        mvall = cpool.tile([P, N_TILES], dt.float32)
        idxfall = cpool.tile([P, N_TILES], dt.float32)
        ltall = cpool.tile([P, N_TILES], dt.float32)
        rtall = cpool.tile([P, N_TILES], dt.float32)
        mv = cpool.tile([P, 8], dt.float32)
        mi = cpool.tile([P, 8], dt.uint32)
        gsc = cpool.tile([P, 38], dt.float32)
        for i in range(N_TILES):
            Vx.max_with_indices(mv, mi, smal[:, i, :])
            Vx.tensor_copy(mvall[:, i:i + 1], mv[:, 0:1])
            Vx.tensor_copy(idxfall[:, i:i + 1], mi[:, 0:1])
        idx2all = cpool.tile([P, N_TILES], dt.float32)
        Vx.tensor_scalar(out=idx2all, in0=idxfall, scalar1=2.0, scalar2=None,
                         op0=Alu.add)
        for i in range(N_TILES):
            Vx.scalar_tensor_tensor(out=gsc, in0=io38,
                                    scalar=idxfall[:, i:i + 1],
                                    in1=esmall[:, i, :],
                                    op0=Alu.is_equal, op1=Alu.mult,
                                    accum_out=ltall[:, i:i + 1])
            Vx.scalar_tensor_tensor(out=gsc, in0=io38,
                                    scalar=idx2all[:, i:i + 1],
                                    in1=esmall[:, i, :],
                                    op0=Alu.is_equal, op1=Alu.mult,
                                    accum_out=rtall[:, i:i + 1])

        den0 = cpool.tile([P, N_TILES], dt.float32)
        den = cpool.tile([P, N_TILES], dt.float32)
        num = cpool.tile([P, N_TILES], dt.float32)
        Vx.tensor_tensor(out=den0, in0=ltall, in1=rtall, op=Alu.add)
        Vx.scalar_tensor_tensor(out=den, in0=mvall, scalar=-2.0,
                                in1=den0, op0=Alu.mult, op1=Alu.add)
        Vx.tensor_tensor(out=num, in0=ltall, in1=rtall, op=Alu.subtract)
        rd = cpool.tile([P, N_TILES], dt.float32)
        scr1 = cpool.tile([P, N_TILES], dt.float32)
        Vx.reciprocal_approx_accurate(out=rd, in_=den, scratch=scr1)
        nr1 = cpool.tile([P, N_TILES], dt.float32)
        Vx.tensor_tensor(out=nr1, in0=den, in1=rd, op=Alu.mult)
        Vx.tensor_scalar(out=nr1, in0=nr1, scalar1=2.0, scalar2=-1.0,
                         op0=Alu.subtract, op1=Alu.mult)  # 2 - den*rd
        Vx.tensor_tensor(out=rd, in0=nr1, in1=rd, op=Alu.mult)
        q = cpool.tile([P, N_TILES], dt.float32)
        Vx.tensor_tensor(out=q, in0=num, in1=rd, op=Alu.mult)
        angall = cpool.tile([P, N_TILES], dt.float32)
        Vx.scalar_tensor_tensor(out=q, in0=q, scalar=0.5, in1=idxfall,
                                op0=Alu.mult, op1=Alu.add)
        Vx.tensor_scalar(out=angall, in0=q, scalar1=CNPI18, scalar2=CPI,
                         op0=Alu.mult, op1=Alu.add)
        # single output DMA: ang_out[i*128 + p] = angall[p, i]
        full = ang_out[:, :]
        out_ap = bass.AP(tensor=full.tensor, offset=full.offset,
                         ap=[[1, P], [P, N_TILES]])
        dma.dma_start(out=out_ap, in_=angall)

    mybir.codegen_inst_isa_subclasses(nc)
    return nc, w34_np


def _run(patch, trace=False):
    _install_birpatch()
    patch = np.ascontiguousarray(np.asarray(patch, dtype=np.float32)
                                 .reshape(B_TOTAL, 32, 32))
    nc, w34_np = _build()
    w34_full = np.ascontiguousarray(np.broadcast_to(w34_np, (P, 1024))
                                    .astype(np.float32))
    in_maps = [
        {"x_in": np.ascontiguousarray(patch[c * B_CORE:(c + 1) * B_CORE]),
         "w34_in": w34_full}
        for c in range(N_CORES)
    ]
    kwargs = {}
    if trace:
        kwargs = dict(trace=True, trace_cores=[0])
    res = run_bass_kernel_spmd(nc, in_maps, list(range(N_CORES)), **kwargs)
    out = np.concatenate(
        [np.asarray(res.results[c]["ang_out"]).reshape(B_CORE)
         for c in range(N_CORES)])
    return out.astype(np.float32), res


def kernel(patch):
    return _run(patch)[0]
